# revision 15
# baseline (speedup 1.0000x reference)
"""LSTM encoder with EOS-freeze for Trainium2, data-parallel over batch on 8 cores.

Strategy (v3)
-------------
Inputs are one-hot, so x @ Wi + b is a row lookup of Wi done entirely on the
host; the looked-up rows are pre-transposed into the z^T layout
[128 part = zfeat%128, 16*tile + b] and streamed from DRAM, then injected into
PSUM with a single identity-stationary matmul per step (h-independent, so it
overlaps the previous step's activation tail).

The recurrent h @ Wh runs on the tensor engine with Wh quantized to fp8-e4m3
(global scale S; device e4m3 reserves exponent 1111 for Inf/NaN so the max
finite magnitude is 240) as 64 [128,128] stationary tiles; fp8 FWL halves the
LoadStationary time vs fp16, which is the recurrence's floor. The moving
operand stays fp16 (h), PSUM accumulates fp32, and the 1/S descale rides the
ACT engine's `scale` parameter.

Matmuls are ordered gate-outer (g, i, f, o): each gate's 16-matmul
accumulation finishes early so its ACT op (subtile deps) runs under the
remaining matmuls; only sigmoid(o) and the final h-multiply trail the last
matmul, covered by the next step's x-injection.

Per-step (c, h) snapshots are written into per-block wide SBUF tiles and
stored to DRAM once per 16-step block (per-step row-wise stores would spend
~1.3us/step of ACT-queue descriptor generation). The frozen value for
sequence b is the snapshot at its first-EOS step, selected during unshard.
"""

import numpy as np

try:
    import concourse  # noqa: F401
except ImportError:
    import sys

    sys.path.insert(0, "/opt/trn_rl_repo")

from contextlib import ExitStack

import ml_dtypes

import concourse.bass as bass  # noqa: F401
import concourse.tile as tile
from concourse import bacc
from concourse import mybir
from concourse.bass import ds
from concourse.bass_utils import run_bass_kernel_spmd

dt = mybir.dt
Alu = mybir.AluOpType
Act = mybir.ActivationFunctionType

EOS_ID = 1
HID = 512
BATCH, SEQ, VOCAB = 128, 256, 1024
GATES = 4 * HID  # 2048
NCORES = 8
BLOC = BATCH // NCORES  # 16 sequences per core
NT = GATES // 128  # 16 feature tiles of z
NK = HID // 128  # 4 contraction chunks
BODY = 16  # steps per For_i iteration
# Device float8e4 is IEEE-style e4m3: exponent 1111 encodes Inf/NaN, so the
# max finite value is 240 (not e4m3fn's 448). Quantize below that.
FP8_MAX = 224.0

# Gate-outer matmul order: g first (longest dependency chain TG->A->c->T->h),
# o last (only sigmoid(o) and the h-mult trail the final matmul).
GATE_ORDER = [3, 0, 1, 2]  # tile groups: g=12-15, i=0-3, f=4-7, o=8-11

# Collect profiling info when True (set by test.py; adds trace overhead).
TRACE = False
LAST_RESULTS = None  # BassKernelResults of the last run, for test.py

_PROGRAM = None
_PROGRAM_KEY = None


def _build_program(inv_scale, seq=SEQ, body=BODY):
    nc = bacc.Bacc("TRN2", debug=False, detect_race_conditions=False)

    wh = nc.declare_dram_parameter("wh", [128, NK * NT * 128], dt.float8e4, isOutput=False)
    ident = nc.declare_dram_parameter("ident", [128, 128], dt.float16, isOutput=False)
    xz = nc.declare_dram_parameter("xz", [(seq + body) * 128, 256], dt.float16, isOutput=False)
    c_traj = nc.declare_dram_parameter("c_traj", [128, seq * 64], dt.float32, isOutput=True)
    h_traj = nc.declare_dram_parameter("h_traj", [128, seq * 64], dt.float16, isOutput=True)

    with tile.TileContext(nc) as tc, ExitStack() as ctx:
        pool = lambda name, bufs, **kw: ctx.enter_context(
            tc.tile_pool(name=name, bufs=bufs, **kw)
        )
        whp = pool("whp", 1)
        xp_pool = pool("xp", 1)
        hp = pool("hp", 1)
        cp = pool("cp", 1)
        zp_pool = pool("zp", 2, space="PSUM")
        sp = pool("sp", 2)
        gp = pool("gp", 2)
        ap_ = pool("ap", 2)
        bp = pool("bp", 2)
        tp = pool("tp", 2)

        wh_sb = whp.tile([128, NK * NT * 128], dt.float8e4, name="wh_sb")
        nc.sync.dma_start(out=wh_sb[:], in_=wh[:, :])
        id_sb = whp.tile([128, 128], dt.float16, name="id_sb")
        nc.sync.dma_start(out=id_sb[:], in_=ident[:, :])

        XT = [xp_pool.tile([128, 256], dt.float16, name=f"xt{s}", tag=f"xt{s}") for s in range(body)]
        # Per-block wide state tiles; step s owns columns [64s, 64s+64).
        H_wide = hp.tile([128, body * 64], dt.float16, name="h_wide")
        C_wide = cp.tile([128, body * 64], dt.float32, name="c_wide")

        nc.gpsimd.memset(H_wide[:], 0.0)
        nc.gpsimd.memset(C_wide[:], 0.0)

        # Block 0's x rows.
        for s in range(body):
            nc.sync.dma_start(out=XT[s][:], in_=xz[s * 128 : (s + 1) * 128, :])

        def step(iv, s):
            sp_prev = 64 * ((s - 1) % body)
            zps = zp_pool.tile([128, 256], dt.float32, name="zps", tag="zpsum")
            # x-part injection: PSUM = X^T via identity-stationary matmul.
            nc.tensor.matmul(
                out=zps[:, 0:256],
                lhsT=id_sb[:],
                rhs=XT[s][:],
                start=True,
                stop=False,
            )
            n = 0
            for gidx, grp in enumerate(GATE_ORDER):
                for k in range(NK):
                    for t in range(4 * grp, 4 * grp + 4):
                        n += 1
                        nc.tensor.matmul(
                            out=zps[:, 16 * t : 16 * t + 16],
                            lhsT=wh_sb[:, (k * NT + t) * 128 : (k * NT + t) * 128 + 128],
                            rhs=H_wide[:, sp_prev + 16 * k : sp_prev + 16 * k + 16],
                            start=False,
                            stop=(n == NK * NT),
                        )
            # Per-gate activations; each starts as soon as its 64 columns'
            # matmuls retire (subtile deps). scale folds away the fp8 scale.
            TG = gp.tile([128, 64], dt.float16, name="TG", tag="TG")
            nc.scalar.activation(out=TG[:], in_=zps[:, 192:256], func=Act.Tanh, scale=inv_scale)
            Si = sp.tile([128, 64], dt.float32, name="Si", tag="Si")
            nc.scalar.activation(out=Si[:], in_=zps[:, 0:64], func=Act.Sigmoid, scale=inv_scale)
            Sf = sp.tile([128, 64], dt.float32, name="Sf", tag="Sf")
            nc.scalar.activation(out=Sf[:], in_=zps[:, 64:128], func=Act.Sigmoid, scale=inv_scale)
            So = sp.tile([128, 64], dt.float32, name="So", tag="So")
            nc.scalar.activation(out=So[:], in_=zps[:, 128:192], func=Act.Sigmoid, scale=inv_scale)
            A = ap_.tile([128, 64], dt.float32, name="A", tag="A")
            nc.vector.tensor_tensor(out=A[:], in0=Si[:], in1=TG[:], op=Alu.mult)
            B = bp.tile([128, 64], dt.float32, name="B", tag="B")
            nc.vector.tensor_tensor(
                out=B[:], in0=Sf[:], in1=C_wide[:, sp_prev : sp_prev + 64], op=Alu.mult
            )
            nc.vector.tensor_tensor(
                out=C_wide[:, 64 * s : 64 * s + 64], in0=A[:], in1=B[:], op=Alu.add
            )
            T = tp.tile([128, 64], dt.float16, name="T", tag="T")
            nc.scalar.activation(out=T[:], in_=C_wide[:, 64 * s : 64 * s + 64], func=Act.Tanh)
            nc.vector.tensor_tensor(
                out=H_wide[:, 64 * s : 64 * s + 64], in0=So[:], in1=T[:], op=Alu.mult
            )

            # Refill this slot with the next block's x rows (xz is padded with
            # one zero block so the final prefetch stays in bounds).
            nc.sync.dma_start(
                out=XT[s][:], in_=xz[ds((iv + body + s) * 128, 128), :]
            )

        with tc.For_i(0, seq, body, hint_engines=(mybir.EngineType.PE,), staggered_reset=True) as iv:
            for s in range(body):
                step(iv, s)
            # One wide store per block instead of per-step row stores.
            nc.sync.dma_start(out=c_traj[:, ds(iv * 64, body * 64)], in_=C_wide[:])
            nc.sync.dma_start(out=h_traj[:, ds(iv * 64, body * 64)], in_=H_wide[:])

    nc.finalize()
    return nc


def _get_program(inv_scale):
    global _PROGRAM, _PROGRAM_KEY
    if _PROGRAM is None or _PROGRAM_KEY != inv_scale:
        _PROGRAM = _build_program(inv_scale)
        _PROGRAM_KEY = inv_scale
    return _PROGRAM


def _prep_host(inputs, Wi, Wh, b):
    tokens = np.argmax(inputs, axis=-1).astype(np.int32)  # [B, T]
    eos = inputs[:, :, EOS_ID] > 0.5
    any_eos = eos.any(axis=1)
    t_star = np.where(any_eos, eos.argmax(axis=1), SEQ - 1).astype(np.int64)

    # Gate reorder (i, f, o, g).
    perm = np.concatenate(
        [np.arange(0, 512), np.arange(512, 1024), np.arange(1536, 2048), np.arange(1024, 1536)]
    )
    Wi_re = (Wi.astype(np.float32) + b.astype(np.float32)[None, :])[:, perm]
    Wh_re = Wh.astype(np.float32)[:, perm]

    scale = FP8_MAX / max(float(np.abs(Wh_re).max()), 1e-30)
    Wh8 = (Wh_re * scale).astype(ml_dtypes.float8_e4m3fn)
    # Partition-major: wh[kr, (k*NT+t)*128 + p] = Wh_re[128k+kr, 128t+p]
    Wh_dev = np.ascontiguousarray(
        Wh8.reshape(NK, 128, NT, 128).transpose(1, 0, 2, 3).reshape(128, NK * NT * 128)
    )
    return tokens, t_star, Wi_re, Wh_dev, scale


def kernel(inputs, Wi, Wh, b):
    global LAST_RESULTS
    inputs = np.asarray(inputs)
    Wi = np.asarray(Wi)
    Wh = np.asarray(Wh)
    b = np.asarray(b)

    tokens, t_star, Wi_re, Wh_dev, scale = _prep_host(inputs, Wi, Wh, b)
    Wi_s = (Wi_re * scale).astype(np.float16)  # [V, 2048], pre-scaled

    in_maps = []
    for n in range(NCORES):
        tokc = tokens[BLOC * n : BLOC * (n + 1)]  # [16, 256]
        Xc = Wi_s[tokc]  # [16, 256, 2048] fp16
        # xz[(s*128)+p, 16t+b] = Xc[b, s, 128t+p]
        xzc = np.zeros(((SEQ + BODY) * 128, 256), np.float16)
        xzc[: SEQ * 128] = (
            Xc.reshape(BLOC, SEQ, NT, 128).transpose(1, 3, 2, 0).reshape(SEQ * 128, 256)
        )
        in_maps.append(
            {
                "wh": Wh_dev,
                "ident": np.eye(128, dtype=np.float16),
                "xz": xzc,
            }
        )

    nc = _get_program(float(1.0 / scale))
    res = run_bass_kernel_spmd(nc, in_maps, list(range(NCORES)), trace=TRACE)
    LAST_RESULTS = res

    c_out = np.zeros((BATCH, HID), np.float32)
    h_out = np.zeros((BATCH, HID), np.float32)
    for n in range(NCORES):
        # traj layout: [128 part, SEQ*64] with col = 64*t + 16*k + b
        ct = res.results[n]["c_traj"].reshape(128, SEQ, 64)
        ht = res.results[n]["h_traj"].reshape(128, SEQ, 64).astype(np.float32)
        for bl in range(BLOC):
            g = BLOC * n + bl
            t = int(t_star[g])
            c_out[g] = ct[:, t, bl::BLOC].T.reshape(HID)
            h_out[g] = ht[:, t, bl::BLOC].T.reshape(HID)
    return (c_out, h_out)


# revision 17
# speedup vs baseline: 1.3214x; 1.3214x over previous
"""LSTM encoder with EOS-freeze for Trainium2, data-parallel over batch on 8 cores.

Strategy (v3)
-------------
Inputs are one-hot, so x @ Wi + b is a row lookup of Wi done entirely on the
host; the looked-up rows are pre-transposed into the z^T layout
[128 part = zfeat%128, 16*tile + b] and streamed from DRAM, then injected into
PSUM with a single identity-stationary matmul per step (h-independent, so it
overlaps the previous step's activation tail).

The recurrent h @ Wh runs on the tensor engine with Wh quantized to fp8-e4m3
(global scale S; device e4m3 reserves exponent 1111 for Inf/NaN so the max
finite magnitude is 240) as 64 [128,128] stationary tiles; fp8 FWL halves the
LoadStationary time vs fp16, which is the recurrence's floor. The moving
operand stays fp16 (h), PSUM accumulates fp32, and the 1/S descale rides the
ACT engine's `scale` parameter.

Matmuls are ordered gate-outer (g, i, f, o): each gate's 16-matmul
accumulation finishes early so its ACT op (subtile deps) runs under the
remaining matmuls; only sigmoid(o) and the final h-multiply trail the last
matmul, covered by the next step's x-injection.

Per-step (c, h) snapshots are written into per-block wide SBUF tiles and
stored to DRAM once per 16-step block (per-step row-wise stores would spend
~1.3us/step of ACT-queue descriptor generation). The frozen value for
sequence b is the snapshot at its first-EOS step, selected during unshard.
"""

import numpy as np

try:
    import concourse  # noqa: F401
except ImportError:
    import sys

    sys.path.insert(0, "/opt/trn_rl_repo")

from contextlib import ExitStack

import ml_dtypes

import concourse.bass as bass  # noqa: F401
import concourse.tile as tile
from concourse import bacc
from concourse import mybir
from concourse.bass import ds
from concourse.bass_utils import run_bass_kernel_spmd

dt = mybir.dt
Alu = mybir.AluOpType
Act = mybir.ActivationFunctionType

EOS_ID = 1
HID = 512
BATCH, SEQ, VOCAB = 128, 256, 1024
GATES = 4 * HID  # 2048
NCORES = 8
BLOC = BATCH // NCORES  # 16 sequences per core
NT = GATES // 128  # 16 feature tiles of z
NK = HID // 128  # 4 contraction chunks
BODY = 16  # steps per For_i iteration
# Device float8e4 is IEEE-style e4m3: exponent 1111 encodes Inf/NaN, so the
# max finite value is 240 (not e4m3fn's 448). Quantize below that.
FP8_MAX = 224.0

# Collect profiling info when True (set by test.py; adds trace overhead).
TRACE = False
LAST_RESULTS = None  # BassKernelResults of the last run, for test.py

_PROGRAM = None
_PROGRAM_KEY = None


def _build_program(inv_scale, seq=SEQ, body=BODY):
    nc = bacc.Bacc("TRN2", debug=False, detect_race_conditions=False)

    wh = nc.declare_dram_parameter("wh", [128, NK * NT * 128], dt.float8e4, isOutput=False)
    ident = nc.declare_dram_parameter("ident", [128, 128], dt.float16, isOutput=False)
    xz = nc.declare_dram_parameter("xz", [(seq + body) * 128, 256], dt.float16, isOutput=False)
    c_traj = nc.declare_dram_parameter("c_traj", [128, seq * 64], dt.float32, isOutput=True)
    h_traj = nc.declare_dram_parameter("h_traj", [128, seq * 64], dt.float16, isOutput=True)

    with tile.TileContext(nc) as tc, ExitStack() as ctx:
        pool = lambda name, bufs, **kw: ctx.enter_context(
            tc.tile_pool(name=name, bufs=bufs, **kw)
        )
        whp = pool("whp", 1)
        xp_pool = pool("xp", 1)
        hp = pool("hp", 1)
        cp = pool("cp", 1)
        zp_pool = pool("zp", 2, space="PSUM")
        sp = pool("sp", 2)
        gp = pool("gp", 2)
        ap_ = pool("ap", 2)
        bp = pool("bp", 2)
        tp = pool("tp", 2)

        wh_sb = whp.tile([128, NK * NT * 128], dt.float8e4, name="wh_sb")
        nc.sync.dma_start(out=wh_sb[:], in_=wh[:, :])
        id_sb = whp.tile([128, 128], dt.float16, name="id_sb")
        nc.sync.dma_start(out=id_sb[:], in_=ident[:, :])

        XT = [xp_pool.tile([128, 256], dt.float16, name=f"xt{s}", tag=f"xt{s}") for s in range(body)]
        # Per-block wide state tiles; step s owns columns [64s, 64s+64).
        H_wide = hp.tile([128, body * 64], dt.float16, name="h_wide")
        C_wide = cp.tile([128, body * 64], dt.float32, name="c_wide")

        nc.gpsimd.memset(H_wide[:], 0.0)
        nc.gpsimd.memset(C_wide[:], 0.0)

        # Block 0's x rows.
        for s in range(body):
            nc.sync.dma_start(out=XT[s][:], in_=xz[s * 128 : (s + 1) * 128, :])

        def step(iv, s):
            sp_prev = 64 * ((s - 1) % body)
            # One PSUM tile per gate -> independent accumulation groups, so a
            # gate's activation starts as soon as its own 16 matmuls retire
            # instead of waiting for the step's full 64-matmul group.
            Z = {}
            for gname in ("zg", "zi", "zf", "zo"):
                Z[gname] = zp_pool.tile([128, 64], dt.float32, name=gname, tag=gname)
            # x-part injections: PSUM = X^T chunk via identity-stationary
            # matmuls (h-independent; overlap the previous step's tail).
            # Host gate layout in XT columns: i=0:64 f=64:128 o=128:192 g=192:256.
            xcol = {"zi": 0, "zf": 64, "zo": 128, "zg": 192}
            for gname in ("zg", "zi", "zf", "zo"):
                nc.tensor.matmul(
                    out=Z[gname][:, 0:64],
                    lhsT=id_sb[:],
                    rhs=XT[s][:, xcol[gname] : xcol[gname] + 64],
                    start=True,
                    stop=False,
                )
            for gname, grp in (("zg", 3), ("zi", 0), ("zf", 1), ("zo", 2)):
                for k in range(NK):
                    for j, t in enumerate(range(4 * grp, 4 * grp + 4)):
                        nc.tensor.matmul(
                            out=Z[gname][:, 16 * j : 16 * j + 16],
                            lhsT=wh_sb[:, (k * NT + t) * 128 : (k * NT + t) * 128 + 128],
                            rhs=H_wide[:, sp_prev + 16 * k : sp_prev + 16 * k + 16],
                            start=False,
                            stop=(k == NK - 1 and j == 3),
                        )
            # Per-gate activations; scale folds away the fp8 scale.
            TG = gp.tile([128, 64], dt.float16, name="TG", tag="TG")
            nc.scalar.activation(out=TG[:], in_=Z["zg"][:, 0:64], func=Act.Tanh, scale=inv_scale)
            Si = sp.tile([128, 64], dt.float32, name="Si", tag="Si")
            nc.scalar.activation(out=Si[:], in_=Z["zi"][:, 0:64], func=Act.Sigmoid, scale=inv_scale)
            Sf = sp.tile([128, 64], dt.float32, name="Sf", tag="Sf")
            nc.scalar.activation(out=Sf[:], in_=Z["zf"][:, 0:64], func=Act.Sigmoid, scale=inv_scale)
            So = sp.tile([128, 64], dt.float32, name="So", tag="So")
            nc.scalar.activation(out=So[:], in_=Z["zo"][:, 0:64], func=Act.Sigmoid, scale=inv_scale)
            A = ap_.tile([128, 64], dt.float32, name="A", tag="A")
            nc.vector.tensor_tensor(out=A[:], in0=Si[:], in1=TG[:], op=Alu.mult)
            B = bp.tile([128, 64], dt.float32, name="B", tag="B")
            nc.vector.tensor_tensor(
                out=B[:], in0=Sf[:], in1=C_wide[:, sp_prev : sp_prev + 64], op=Alu.mult
            )
            nc.vector.tensor_tensor(
                out=C_wide[:, 64 * s : 64 * s + 64], in0=A[:], in1=B[:], op=Alu.add
            )
            T = tp.tile([128, 64], dt.float16, name="T", tag="T")
            nc.scalar.activation(out=T[:], in_=C_wide[:, 64 * s : 64 * s + 64], func=Act.Tanh)
            nc.vector.tensor_tensor(
                out=H_wide[:, 64 * s : 64 * s + 64], in0=So[:], in1=T[:], op=Alu.mult
            )

            # Refill this slot with the next block's x rows (xz is padded with
            # one zero block so the final prefetch stays in bounds).
            nc.sync.dma_start(
                out=XT[s][:], in_=xz[ds((iv + body + s) * 128, 128), :]
            )

        with tc.For_i(0, seq, body, hint_engines=(mybir.EngineType.PE,), staggered_reset=True) as iv:
            for s in range(body):
                step(iv, s)
            # One wide store per block instead of per-step row stores.
            nc.sync.dma_start(out=c_traj[:, ds(iv * 64, body * 64)], in_=C_wide[:])
            nc.sync.dma_start(out=h_traj[:, ds(iv * 64, body * 64)], in_=H_wide[:])

    nc.finalize()
    return nc


def _get_program(inv_scale):
    global _PROGRAM, _PROGRAM_KEY
    if _PROGRAM is None or _PROGRAM_KEY != inv_scale:
        _PROGRAM = _build_program(inv_scale)
        _PROGRAM_KEY = inv_scale
    return _PROGRAM


def _prep_host(inputs, Wi, Wh, b):
    tokens = np.argmax(inputs, axis=-1).astype(np.int32)  # [B, T]
    eos = inputs[:, :, EOS_ID] > 0.5
    any_eos = eos.any(axis=1)
    t_star = np.where(any_eos, eos.argmax(axis=1), SEQ - 1).astype(np.int64)

    # Gate reorder (i, f, o, g).
    perm = np.concatenate(
        [np.arange(0, 512), np.arange(512, 1024), np.arange(1536, 2048), np.arange(1024, 1536)]
    )
    Wi_re = (Wi.astype(np.float32) + b.astype(np.float32)[None, :])[:, perm]
    Wh_re = Wh.astype(np.float32)[:, perm]

    scale = FP8_MAX / max(float(np.abs(Wh_re).max()), 1e-30)
    Wh8 = (Wh_re * scale).astype(ml_dtypes.float8_e4m3fn)
    # Partition-major: wh[kr, (k*NT+t)*128 + p] = Wh_re[128k+kr, 128t+p]
    Wh_dev = np.ascontiguousarray(
        Wh8.reshape(NK, 128, NT, 128).transpose(1, 0, 2, 3).reshape(128, NK * NT * 128)
    )
    return tokens, t_star, Wi_re, Wh_dev, scale


def kernel(inputs, Wi, Wh, b):
    global LAST_RESULTS
    inputs = np.asarray(inputs)
    Wi = np.asarray(Wi)
    Wh = np.asarray(Wh)
    b = np.asarray(b)

    tokens, t_star, Wi_re, Wh_dev, scale = _prep_host(inputs, Wi, Wh, b)
    Wi_s = (Wi_re * scale).astype(np.float16)  # [V, 2048], pre-scaled

    in_maps = []
    for n in range(NCORES):
        tokc = tokens[BLOC * n : BLOC * (n + 1)]  # [16, 256]
        Xc = Wi_s[tokc]  # [16, 256, 2048] fp16
        # xz[(s*128)+p, 16t+b] = Xc[b, s, 128t+p]
        xzc = np.zeros(((SEQ + BODY) * 128, 256), np.float16)
        xzc[: SEQ * 128] = (
            Xc.reshape(BLOC, SEQ, NT, 128).transpose(1, 3, 2, 0).reshape(SEQ * 128, 256)
        )
        in_maps.append(
            {
                "wh": Wh_dev,
                "ident": np.eye(128, dtype=np.float16),
                "xz": xzc,
            }
        )

    nc = _get_program(float(1.0 / scale))
    res = run_bass_kernel_spmd(nc, in_maps, list(range(NCORES)), trace=TRACE)
    LAST_RESULTS = res

    c_out = np.zeros((BATCH, HID), np.float32)
    h_out = np.zeros((BATCH, HID), np.float32)
    for n in range(NCORES):
        # traj layout: [128 part, SEQ*64] with col = 64*t + 16*k + b
        ct = res.results[n]["c_traj"].reshape(128, SEQ, 64)
        ht = res.results[n]["h_traj"].reshape(128, SEQ, 64).astype(np.float32)
        for bl in range(BLOC):
            g = BLOC * n + bl
            t = int(t_star[g])
            c_out[g] = ct[:, t, bl::BLOC].T.reshape(HID)
            h_out[g] = ht[:, t, bl::BLOC].T.reshape(HID)
    return (c_out, h_out)
